# revision 17
# baseline (speedup 1.0000x reference)
"""Causal self-attention (RoPE + QK-RMSNorm, GQA 16q/8kv) Trainium2 Bass kernel.

Sharding: 8 cores = 2 batch x 4 tensor-parallel. Core c handles batch b=c//4 and
q-heads [4*tp, 4*tp+4), kv-heads [2*tp, 2*tp+2) where tp=c%4. Each core returns a
partial (T, C) output = O_heads @ wo[rows of its heads]; host sums the 4 partials
per batch (the "all-reduce after c_proj").

v7 schedule: all 4 projection chunks run first (dense back-to-back matmuls keep
the PE HAM-warm), then the 4 attention spans. Output projections of span s ride
as PE fillers inside span s+1's score stream. Within a span, the two q-heads
sharing a kv-head advance in interleaved waves; their scores land in one
2-bank PSUM tile so a single exp call covers both heads (the scalar engine is
the span-phase pacer). Rope temporaries are bf16 (2x DVE rate); V copies and
diagonal masks run on the otherwise-idle GPSIMD. DMA transfers are issued
few-and-large on both HWDGE queues (issue rate, not bandwidth, binds startup).
"""
import sys
import math

sys.path.insert(0, "/opt/trn_rl_repo")

import numpy as np
import ml_dtypes
import concourse.bacc as bacc
import concourse.mybir as mybir
import concourse.tile as tile
from concourse.bass_utils import run_bass_kernel_spmd

P = 128
T = 2048
C = 2048
KO = C // P          # 16 contraction tiles
D = 128              # head dim
NQ = 4               # q heads per core
NK = 2               # kv heads per core
NF = NQ + NK         # 6 rope/rms feature blocks (4 q + 2 k)
FQ = NQ * D          # 512
FK = NK * D          # 256
TCH = 512            # chunk / span size
NCHUNK = T // TCH    # 4
SPAN = 512
KB = T // P          # 16 key blocks
SCALE = 1.0 / math.sqrt(D)
LAG = 2              # AV/sum lag (in waves) behind the score stream

f32 = mybir.dt.float32
bf16 = mybir.dt.bfloat16

AF = mybir.ActivationFunctionType


def build():
    nc = bacc.Bacc("TRN2", target_bir_lowering=False)
    # pre-tiled DMA-contiguous input layouts (see _host_inputs)
    xTt = nc.dram_tensor("xTt", (NCHUNK, 4, P, 4, TCH), bf16, kind="ExternalInput")
    wqt = nc.dram_tensor("wqt", (NQ, P, KO, D), bf16, kind="ExternalInput")
    wkt = nc.dram_tensor("wkt", (NK, P, KO, D), bf16, kind="ExternalInput")
    wvt = nc.dram_tensor("wvt", (P, KO, FK), bf16, kind="ExternalInput")
    wot = nc.dram_tensor("wot", (NQ, P, C), bf16, kind="ExternalInput")
    cc = nc.dram_tensor("cc", (P, T), bf16, kind="ExternalInput")    # [cos; cos]
    ss = nc.dram_tensor("ss", (P, T), bf16, kind="ExternalInput")    # [sin; -sin]
    mask = nc.dram_tensor("mask", (P, P), bf16, kind="ExternalInput")  # [k, qq] = qq>=k
    y = nc.dram_tensor("y", (T, C), bf16, kind="ExternalOutput")

    with tile.TileContext(nc) as tc:
        with (
            tc.tile_pool(name="persist", bufs=1) as persist,
            tc.tile_pool(name="otp", bufs=2) as otp,
            tc.tile_pool(name="xp", bufs=2) as xp,
            tc.tile_pool(name="tpf", bufs=3) as tpf,
            tc.tile_pool(name="tpw", bufs=6) as tpw,
            tc.tile_pool(name="tps", bufs=3) as tps,
            tc.tile_pool(name="sqp", bufs=7) as sqp,
            tc.tile_pool(name="rstdp", bufs=8) as rstdp,
            tc.tile_pool(name="tpt", bufs=4) as tpt,
            tc.tile_pool(name="tpy", bufs=6) as tpy,
            tc.tile_pool(name="ps_mm", bufs=2, space="PSUM") as ps_mm,
            tc.tile_pool(name="ps_st", bufs=1, space="PSUM") as ps_st,
            tc.tile_pool(name="ps_ot", bufs=2, space="PSUM") as ps_ot,
            tc.tile_pool(name="ps_sum", bufs=2, space="PSUM") as ps_sum,
        ):
            qk_rt = persist.tile([P, NF, T], bf16, tag="qk_rt")   # roped+normed qT/kT
            v_sb = persist.tile([P, KB, FK], bf16, tag="v_sb")    # V natural [t-part, kb, feat]
            cc_sb = persist.tile([P, T], bf16, tag="cc_sb")
            ss_sb = persist.tile([P, T], bf16, tag="ss_sb")
            mask_sb = persist.tile([P, P], bf16, tag="mask_sb")
            ones_col = persist.tile([P, 1], bf16, tag="ones_col")    # sums lhsT
            ones_row = persist.tile([1, P], bf16, tag="ones_row")    # bcast lhsT
            ones_f32 = persist.tile([P, 1], f32, tag="ones_f32")
            ones_row_f32 = persist.tile([1, P], f32, tag="ones_row_f32")
            wq_sb = persist.tile([P, NQ, KO, D], bf16, tag="wq_sb")
            wk_sb = persist.tile([P, NK, KO, D], bf16, tag="wk_sb")
            wv_sb = persist.tile([P, KO, FK], bf16, tag="wv_sb")
            wo_sb = persist.tile([P, NQ, C], bf16, tag="wo_sb")

            xts = [None] * NCHUNK

            def issue_x(c, engs):
                """Four 512KB group-transfers; HWDGE issue rate binds, so keep
                the count low."""
                xt = xp.tile([P, KO, TCH], bf16, tag="xt")
                for g in range(4):
                    eng = engs[g % len(engs)]
                    eng.dma_start(xt[:, 4 * g : 4 * (g + 1), :], xTt[c, g])
                xts[c] = xt

            # -- startup DMA schedule: first-needed first, split across both
            #    HWDGE issue queues.
            nc.sync.dma_start(wk_sb[:, 0], wkt[0])
            nc.scalar.dma_start(wk_sb[:, 1], wkt[1])
            issue_x(0, (nc.sync, nc.scalar))
            nc.sync.dma_start(cc_sb[:], cc[:, :])
            nc.scalar.dma_start(ss_sb[:], ss[:, :])
            for fb in range(NQ):
                eng = nc.sync if fb % 2 == 0 else nc.scalar
                eng.dma_start(wq_sb[:, fb], wqt[fb])
            nc.sync.dma_start(wv_sb[:], wvt[:, :, :])
            nc.scalar.dma_start(mask_sb[:], mask[:, :])
            nc.vector.memset(ones_f32[:], 1.0)
            nc.vector.memset(ones_row_f32[:], 1.0)
            nc.vector.tensor_copy(ones_col[:], ones_f32[:])
            nc.vector.tensor_copy(ones_row[:], ones_row_f32[:])

            # ---------------- chunk projection ----------------
            def emit_chunk(c, deferred_in):
                """Project chunk c. `deferred_in`: apply thunks from the
                previous chunk. Returns this chunk's deferred thunks."""
                t0 = c * TCH
                xt = xts[c]
                segs = [None] * NF
                rstds = {}
                dq = list(deferred_in)

                def pop_deferred():
                    if dq:
                        dq.pop(0)()

                swps = {}
                tmpas = {}

                def emit_fb_a(fb):
                    """Projection matmuls + rope prologue: raw copy, swap-DMA
                    issue, cos product. Sin-side products batch in emit_fb_b
                    once all swaps are in flight, so neither the DVE nor the
                    scalar queue head-blocks on DMA latency."""
                    if fb < NQ:
                        w_ap = wq_sb[:, fb]
                    else:
                        w_ap = wk_sb[:, fb - NQ]
                    pqk = ps_mm.tile([P, TCH], f32, tag="ps_mm")
                    for ko in range(KO):
                        nc.tensor.matmul(
                            pqk[:], w_ap[:, ko], xt[:, ko, :],
                            start=(ko == 0), stop=(ko == KO - 1),
                        )
                    raw = tpf.tile([P, TCH], bf16, tag="raw")
                    nc.scalar.activation(raw[:], pqk[:], AF.Copy)
                    # half swaps, one per HWDGE queue (bf16: 64KB each)
                    swp = tpw.tile([P, TCH], bf16, tag="swp")
                    nc.sync.dma_start(swp[0:64, :], raw[64:128, :])
                    nc.scalar.dma_start(swp[64:128, :], raw[0:64, :])
                    tmpa = tpw.tile([P, TCH], bf16, tag="tmpa")
                    nc.vector.tensor_mul(tmpa[:], raw[:], cc_sb[:, t0 : t0 + TCH])
                    swps[fb] = swp
                    tmpas[fb] = tmpa

                def emit_fb_b(fb):
                    tmpb = tpf.tile([P, TCH], bf16, tag="tmpb")
                    nc.vector.tensor_mul(tmpb[:], swps[fb][:], ss_sb[:, t0 : t0 + TCH])
                    seg = qk_rt[:, fb, t0 : t0 + TCH]
                    nc.vector.tensor_add(seg, tmpas[fb][:], tmpb[:])
                    sq = sqp.tile([P, TCH], bf16, tag="sq")
                    nc.vector.tensor_mul(sq[:], seg, seg)
                    segs[fb] = (seg, sq)

                def emit_stat(fb):
                    pms = ps_sum.tile([1, TCH], f32, tag="ps_sum")
                    nc.tensor.matmul(pms[:], ones_col[:], segs[fb][1][:], start=True, stop=True)
                    # rstd = 1/sqrt(ms) = sqrt(D / pms); eps negligible vs ms
                    inv = tps.tile([1, TCH], f32, tag="inv")
                    nc.vector.reciprocal_approx_fast(inv[:], pms[:])
                    rstd = rstdp.tile([1, TCH], bf16, tag="rstd")
                    nc.scalar.activation(rstd[:], inv[:], AF.Sqrt, scale=float(D))
                    rstds[fb] = rstd

                def emit_apply(fb):
                    pb = ps_mm.tile([P, TCH], f32, tag="ps_mm")
                    nc.tensor.matmul(pb[:], ones_row[:], rstds[fb][:], start=True, stop=True)
                    seg = segs[fb][0]
                    nc.vector.tensor_mul(seg, seg, pb[:])

                def emit_v(tb):
                    pv = ps_mm.tile([P, TCH], f32, tag="ps_mm")
                    for ko in range(KO):
                        nc.tensor.matmul(
                            pv[:, :FK],
                            xt[:, ko, tb * P : (tb + 1) * P],
                            wv_sb[:, ko, :],
                            start=(ko == 0), stop=(ko == KO - 1),
                        )
                    nc.vector.tensor_copy(
                        v_sb[:, c * (TCH // P) + tb, :], pv[:, :FK]
                    )

                # dense fb block first (max slack for the swap-DMA chains),
                # then V blocks with the stat/apply chains interleaved
                for fb in (4, 5, 0, 1, 2, 3):
                    emit_fb_a(fb)
                pop_deferred()                      # prev apply2
                pop_deferred()                      # prev apply3
                if c + 1 < NCHUNK:
                    issue_x(c + 1, (nc.sync,))
                if c == 1:
                    for h in range(NQ):
                        eng = nc.sync if h % 2 == 0 else nc.scalar
                        eng.dma_start(wo_sb[:, h], wot[h])
                for fb in (4, 5, 0, 1, 2, 3):
                    emit_fb_b(fb)
                emit_v(0)
                emit_stat(4)
                emit_v(1)
                emit_stat(5)
                emit_stat(0)
                emit_v(2)
                emit_stat(1)
                emit_apply(4)
                emit_apply(5)
                emit_v(3)
                emit_stat(2)
                emit_apply(0)
                emit_stat(3)
                emit_apply(1)
                # only the (table-free) applies defer into the next phase, so
                # no Sqrt table load ever lands inside the span exp stream
                deferred = [
                    lambda: emit_apply(2),
                    lambda: emit_apply(3),
                ]
                return deferred

            # ---------------- attention span ----------------
            def emit_norm(ot_t, h, ot_ps, rec_r):
                bc = ps_mm.tile([P, SPAN], f32, tag="ps_mm")
                nc.tensor.matmul(bc[:], ones_row[:], rec_r[:], start=True, stop=True)
                bc_sb = tps.tile([P, SPAN], f32, tag="bc_sb")
                nc.vector.tensor_copy(bc_sb[:], bc[:])
                nc.vector.tensor_mul(ot_t[:, h, :], ot_ps[:], bc_sb[:])

            def emit_span(s, fillers):
                """Attention for q-span s. The two q-heads sharing a kv-head
                advance in interleaved waves; one 2-bank PSUM score tile per
                wave feeds a single batched exp. `fillers`: independent PE
                thunks popped one per wave. Returns (ot_t, deferred norms)."""
                q0 = s * SPAN
                nkb = 4 * s + 4
                ot_t = otp.tile([P, NQ, SPAN], bf16, tag="ot_t")
                pending = []

                for j in range(NK):  # kv head = head pair
                    # free previous pair's ot banks before this pair's AVs;
                    # a filler ahead of each norm covers the DVE rec latency
                    while pending:
                        if fillers:
                            fillers.pop(0)()
                        emit_norm(ot_t, *pending.pop(0))
                    hs = (2 * j, 2 * j + 1)
                    ot_ps = {h: ps_ot.tile([P, SPAN], f32, tag="ot_ps", name="ot_ps")
                             for h in hs}
                    sum_ps = {h: ps_sum.tile([1, SPAN], f32, tag="ps_sum", name="sum_ps")
                              for h in hs}
                    queue = []

                    def flush_one():
                        kb, off, vq, pt2 = queue.pop(0)
                        for i, h in enumerate(hs):
                            nc.tensor.matmul(
                                ot_ps[h][:, off:],
                                v_sb[:, kb, j * D : (j + 1) * D],
                                pt2[:, i, :vq],
                                start=(kb == 0), stop=(kb == nkb - 1),
                                skip_group_check=True,
                            )
                            nc.tensor.matmul(
                                sum_ps[h][:, off:],
                                ones_col[:],
                                pt2[:, i, :vq],
                                start=(kb == 0), stop=(kb == nkb - 1),
                                skip_group_check=True,
                            )

                    for kb in range(nkb):
                        r = kb - 4 * s           # >=0: diagonal block group
                        off = P * r if r > 0 else 0
                        vq = SPAN - off
                        st2 = ps_st.tile([P, 2, SPAN], f32, tag="st2")
                        for i, h in enumerate(hs):
                            nc.tensor.matmul(
                                st2[:, i, :vq],
                                qk_rt[:, NQ + j, kb * P : (kb + 1) * P],
                                qk_rt[:, h, q0 + off : q0 + SPAN],
                                start=True, stop=True,
                            )
                        pt2 = tpt.tile([P, 2, SPAN], bf16, tag="pt2")
                        nc.scalar.activation(pt2[:, :, :vq], st2[:, :, :vq],
                                             AF.Exp, scale=SCALE)
                        if r >= 0:
                            for i in range(2):
                                nc.gpsimd.tensor_mul(
                                    pt2[:, i, :P], pt2[:, i, :P], mask_sb[:])
                        queue.append((kb, off, vq, pt2))
                        while queue and queue[0][0] <= kb - LAG:
                            flush_one()
                        if fillers:
                            fillers.pop(0)()
                    while queue:
                        flush_one()
                    # softmax denominators -> reciprocal on DVE; the PE
                    # broadcast is deferred (next pair / next span's stream)
                    for h in hs:
                        rec = tps.tile([1, SPAN], f32, tag="rec")
                        nc.vector.reciprocal_approx_fast(rec[:], sum_ps[h][:])
                        rec_r = tps.tile([1, SPAN], bf16, tag="rec_r")
                        nc.vector.tensor_copy(rec_r[:], rec[:])
                        pending.append((h, ot_ps[h], rec_r))
                # drain any fillers that didn't fit in the wave slots
                while fillers:
                    fillers.pop(0)()
                return ot_t, pending

            def proj_thunks(c, ot_t, split_dma=False):
                """Output projection for span c as independent PE thunks."""
                def one(tb, nch):
                    yps = ps_mm.tile([P, 512], f32, tag="ps_mm")
                    for h in range(NQ):
                        nc.tensor.matmul(
                            yps[:],
                            ot_t[:, h, tb * P : (tb + 1) * P],
                            wo_sb[:, h, nch * 512 : (nch + 1) * 512],
                            start=(h == 0), stop=(h == NQ - 1),
                        )
                    ysb = tpy.tile([P, 512], bf16, tag="ysb")
                    nc.vector.tensor_copy(ysb[:], yps[:])
                    rows = slice((4 * c + tb) * P, (4 * c + tb + 1) * P)
                    if split_dma:
                        # halve the final transfers so the kernel tail isn't
                        # gated by one long DMA
                        nc.sync.dma_start(
                            y[rows, nch * 512 : nch * 512 + 256], ysb[:, :256])
                        nc.scalar.dma_start(
                            y[rows, nch * 512 + 256 : (nch + 1) * 512], ysb[:, 256:])
                    else:
                        nc.sync.dma_start(
                            y[rows, nch * 512 : (nch + 1) * 512], ysb[:])
                return [lambda tb=tb, nch=nch: one(tb, nch)
                        for tb in range(4) for nch in range(C // 512)]

            # ---------------- program ----------------
            # chunks first (dense PE, HAM-warm); x prefetch happens inside
            # each chunk, after its swap-DMA issues
            d = emit_chunk(0, [])
            d = emit_chunk(1, d)
            d = emit_chunk(2, d)
            d = emit_chunk(3, d)

            # spans; span s-1's output projection rides in span s's stream.
            # chunk3's deferred applies pad a wave into span 0 so their rstd
            # chains are ready.
            noop = lambda: None
            d = [noop] + d[:1] + [noop] + d[1:]
            ot0, n0 = emit_span(0, d)
            f1 = [lambda n=n: emit_norm(ot0, *n) for n in n0] + proj_thunks(0, ot0)
            ot1, n1 = emit_span(1, f1)
            f2 = [lambda n=n: emit_norm(ot1, *n) for n in n1] + proj_thunks(1, ot1)
            ot2, n2 = emit_span(2, f2)
            f3 = [lambda n=n: emit_norm(ot2, *n) for n in n2] + proj_thunks(2, ot2)
            ot3, n3 = emit_span(3, f3)
            for n in n3:
                emit_norm(ot3, *n)
            for t in proj_thunks(3, ot3, split_dma=True):
                t()
    nc.compile()
    return nc


_NC_CACHE = None


def _get_nc():
    global _NC_CACHE
    if _NC_CACHE is None:
        _NC_CACHE = build()
    return _NC_CACHE


def _host_inputs(x, cos, sin, wq, wk, wv, wo):
    """Build the 8 per-core input maps with DMA-contiguous pre-tiled layouts."""
    bft = ml_dtypes.bfloat16
    cosT = np.ascontiguousarray(cos[0, :, 0, :].T).astype(np.float32)  # (64, T)
    sinT = np.ascontiguousarray(sin[0, :, 0, :].T).astype(np.float32)
    cc = np.ascontiguousarray(np.concatenate([cosT, cosT], axis=0)).astype(bft)  # (128, T)
    ss = np.ascontiguousarray(np.concatenate([sinT, -sinT], axis=0)).astype(bft)
    # mask[k, qq] = 1 if qq >= k (within the 128-wide diagonal sub-block)
    qq = np.arange(P)[None, :]
    kk = np.arange(P)[:, None]
    mask = np.ascontiguousarray((qq >= kk).astype(bft))  # (128, 128)

    # xTt[c, g, p, kk, t] = x[b][c*TCH+t, (4g+kk)*P+p]
    xTts = []
    for b in range(2):
        xb = x[b].astype(bft)                            # (T, C)
        a = xb.reshape(NCHUNK, TCH, 4, 4, P)             # [c, t, g, kk, p]
        xTts.append(np.ascontiguousarray(a.transpose(0, 2, 4, 3, 1)))

    wq16 = wq.astype(bft)
    wk16 = wk.astype(bft)
    wv16 = wv.astype(bft)
    wo16 = wo.astype(bft)
    in_maps = []
    for core in range(8):
        b, tp = divmod(core, 4)
        wq_s = wq16[:, tp * FQ : (tp + 1) * FQ]     # (C, FQ)
        wk_s = wk16[:, tp * FK : (tp + 1) * FK]     # (C, FK)
        wv_s = wv16[:, tp * FK : (tp + 1) * FK]
        wo_s = wo16[tp * FQ : (tp + 1) * FQ, :]     # (FQ, C)
        # wqt[fb, p, ko, d] = wq_s[ko*P+p, fb*D+d]
        a = wq_s.reshape(KO, P, NQ, D)
        wqt = np.ascontiguousarray(a.transpose(2, 1, 0, 3))          # (NQ, P, KO, D)
        # wkt[kh, p, ko, d] = wk_s[ko*P+p, kh*D+d]
        a = wk_s.reshape(KO, P, NK, D)
        wkt = np.ascontiguousarray(a.transpose(2, 1, 0, 3))          # (NK, P, KO, D)
        # wvt[p, ko, f] = wv_s[ko*P+p, f]
        a = wv_s.reshape(KO, P, FK)
        wvt = np.ascontiguousarray(a.transpose(1, 0, 2))             # (P, KO, FK)
        # wot[h, p, n] = wo_s[h*D+p, n]
        wot = np.ascontiguousarray(wo_s.reshape(NQ, P, C))
        in_maps.append(
            {
                "xTt": xTts[b],
                "wqt": wqt,
                "wkt": wkt,
                "wvt": wvt,
                "wot": wot,
                "cc": cc,
                "ss": ss,
                "mask": mask,
            }
        )
    return in_maps


def kernel(x, cos, sin, wq, wk, wv, wo, trace=False):
    x = np.asarray(x, dtype=np.float32)
    cos = np.asarray(cos, dtype=np.float32)
    sin = np.asarray(sin, dtype=np.float32)
    wq = np.asarray(wq, dtype=np.float32)
    wk = np.asarray(wk, dtype=np.float32)
    wv = np.asarray(wv, dtype=np.float32)
    wo = np.asarray(wo, dtype=np.float32)

    nc = _get_nc()
    in_maps = _host_inputs(x, cos, sin, wq, wk, wv, wo)
    res = run_bass_kernel_spmd(nc, in_maps, core_ids=list(range(8)), trace=trace)
    out = np.zeros((2, T, C), dtype=np.float32)
    for c in range(8):
        b = c // 4
        out[b] += res.results[c]["y"].astype(np.float32)
    if trace:
        return out, res
    return out


# revision 19
# speedup vs baseline: 1.0184x; 1.0184x over previous
"""Causal self-attention (RoPE + QK-RMSNorm, GQA 16q/8kv) Trainium2 Bass kernel.

Sharding: 8 cores = 2 batch x 4 tensor-parallel. Core c handles batch b=c//4 and
q-heads [4*tp, 4*tp+4), kv-heads [2*tp, 2*tp+2) where tp=c%4. Each core returns a
partial (T, C) output = O_heads @ wo[rows of its heads]; host sums the 4 partials
per batch (the "all-reduce after c_proj").

v7 schedule: all 4 projection chunks run first (dense back-to-back matmuls keep
the PE HAM-warm), then the 4 attention spans. Output projections of span s ride
as PE fillers inside span s+1's score stream. Within a span, the two q-heads
sharing a kv-head advance in interleaved waves; their scores land in one
2-bank PSUM tile so a single exp call covers both heads (the scalar engine is
the span-phase pacer). Rope temporaries are bf16 (2x DVE rate); V copies and
diagonal masks run on the otherwise-idle GPSIMD. DMA transfers are issued
few-and-large on both HWDGE queues (issue rate, not bandwidth, binds startup).
"""
import sys
import math

sys.path.insert(0, "/opt/trn_rl_repo")

import numpy as np
import ml_dtypes
import concourse.bacc as bacc
import concourse.mybir as mybir
import concourse.tile as tile
from concourse.bass_utils import run_bass_kernel_spmd

P = 128
T = 2048
C = 2048
KO = C // P          # 16 contraction tiles
D = 128              # head dim
NQ = 4               # q heads per core
NK = 2               # kv heads per core
NF = NQ + NK         # 6 rope/rms feature blocks (4 q + 2 k)
FQ = NQ * D          # 512
FK = NK * D          # 256
TCH = 512            # chunk / span size
NCHUNK = T // TCH    # 4
SPAN = 512
KB = T // P          # 16 key blocks
SCALE = 1.0 / math.sqrt(D)
LAG = 2              # AV/sum lag (in waves) behind the score stream

f32 = mybir.dt.float32
bf16 = mybir.dt.bfloat16

AF = mybir.ActivationFunctionType


def build():
    nc = bacc.Bacc("TRN2", target_bir_lowering=False)
    # pre-tiled DMA-contiguous input layouts (see _host_inputs)
    xTt = nc.dram_tensor("xTt", (NCHUNK, 4, P, 4, TCH), bf16, kind="ExternalInput")
    wqt = nc.dram_tensor("wqt", (NQ, P, KO, D), bf16, kind="ExternalInput")
    wkt = nc.dram_tensor("wkt", (NK, P, KO, D), bf16, kind="ExternalInput")
    wvt = nc.dram_tensor("wvt", (P, KO, FK), bf16, kind="ExternalInput")
    wot = nc.dram_tensor("wot", (NQ, P, C), bf16, kind="ExternalInput")
    cc = nc.dram_tensor("cc", (P, T), bf16, kind="ExternalInput")    # [cos; cos]
    ss = nc.dram_tensor("ss", (P, T), bf16, kind="ExternalInput")    # [sin; -sin]
    mask = nc.dram_tensor("mask", (P, P), bf16, kind="ExternalInput")  # [k, qq] = qq>=k
    y = nc.dram_tensor("y", (T, C), bf16, kind="ExternalOutput")

    with tile.TileContext(nc) as tc:
        with (
            tc.tile_pool(name="persist", bufs=1) as persist,
            tc.tile_pool(name="otp", bufs=2) as otp,
            tc.tile_pool(name="xp", bufs=2) as xp,
            tc.tile_pool(name="tpf", bufs=3) as tpf,
            tc.tile_pool(name="tpw", bufs=6) as tpw,
            tc.tile_pool(name="tps", bufs=3) as tps,
            tc.tile_pool(name="sqp", bufs=7) as sqp,
            tc.tile_pool(name="rstdp", bufs=8) as rstdp,
            tc.tile_pool(name="tpt", bufs=4) as tpt,
            tc.tile_pool(name="tpy", bufs=6) as tpy,
            tc.tile_pool(name="ps_mm", bufs=2, space="PSUM") as ps_mm,
            tc.tile_pool(name="ps_st", bufs=1, space="PSUM") as ps_st,
            tc.tile_pool(name="ps_ot", bufs=2, space="PSUM") as ps_ot,
            tc.tile_pool(name="ps_sum", bufs=2, space="PSUM") as ps_sum,
        ):
            qk_rt = persist.tile([P, NF, T], bf16, tag="qk_rt")   # roped+normed qT/kT
            v_sb = persist.tile([P, KB, FK], bf16, tag="v_sb")    # V natural [t-part, kb, feat]
            cc_sb = persist.tile([P, T], bf16, tag="cc_sb")
            ss_sb = persist.tile([P, T], bf16, tag="ss_sb")
            mask_sb = persist.tile([P, P], bf16, tag="mask_sb")
            ones_col = persist.tile([P, 1], bf16, tag="ones_col")    # sums lhsT
            ones_row = persist.tile([1, P], bf16, tag="ones_row")    # bcast lhsT
            ones_f32 = persist.tile([P, 1], f32, tag="ones_f32")
            ones_row_f32 = persist.tile([1, P], f32, tag="ones_row_f32")
            wq_sb = persist.tile([P, NQ, KO, D], bf16, tag="wq_sb")
            wk_sb = persist.tile([P, NK, KO, D], bf16, tag="wk_sb")
            wv_sb = persist.tile([P, KO, FK], bf16, tag="wv_sb")
            wo_sb = persist.tile([P, NQ, C], bf16, tag="wo_sb")

            xts = [None] * NCHUNK

            def issue_x(c, engs):
                """Four 512KB group-transfers; HWDGE issue rate binds, so keep
                the count low."""
                xt = xp.tile([P, KO, TCH], bf16, tag="xt")
                for g in range(4):
                    eng = engs[g % len(engs)]
                    eng.dma_start(xt[:, 4 * g : 4 * (g + 1), :], xTt[c, g])
                xts[c] = xt

            # -- startup DMA schedule: first-needed first, split across both
            #    HWDGE issue queues.
            nc.sync.dma_start(wk_sb[:, 0], wkt[0])
            nc.scalar.dma_start(wk_sb[:, 1], wkt[1])
            issue_x(0, (nc.sync, nc.scalar))
            nc.sync.dma_start(cc_sb[:], cc[:, :])
            nc.scalar.dma_start(ss_sb[:], ss[:, :])
            for fb in range(NQ):
                eng = nc.sync if fb % 2 == 0 else nc.scalar
                eng.dma_start(wq_sb[:, fb], wqt[fb])
            nc.sync.dma_start(wv_sb[:], wvt[:, :, :])
            nc.scalar.dma_start(mask_sb[:], mask[:, :])
            nc.vector.memset(ones_f32[:], 1.0)
            nc.vector.memset(ones_row_f32[:], 1.0)
            nc.vector.tensor_copy(ones_col[:], ones_f32[:])
            nc.vector.tensor_copy(ones_row[:], ones_row_f32[:])

            # ---------------- chunk projection ----------------
            def emit_chunk(c, deferred_in):
                """Project chunk c. `deferred_in`: apply thunks from the
                previous chunk. Returns this chunk's deferred thunks."""
                t0 = c * TCH
                xt = xts[c]
                segs = [None] * NF
                rstds = {}
                dq = list(deferred_in)

                def pop_deferred():
                    if dq:
                        dq.pop(0)()

                swps = {}
                tmpas = {}

                def emit_fb_a(fb):
                    """Projection matmuls + rope prologue: raw copy, swap-DMA
                    issue, cos product. Sin-side products batch in emit_fb_b
                    once all swaps are in flight, so neither the DVE nor the
                    scalar queue head-blocks on DMA latency."""
                    if fb < NQ:
                        w_ap = wq_sb[:, fb]
                    else:
                        w_ap = wk_sb[:, fb - NQ]
                    pqk = ps_mm.tile([P, TCH], f32, tag="ps_mm")
                    for ko in range(KO):
                        nc.tensor.matmul(
                            pqk[:], w_ap[:, ko], xt[:, ko, :],
                            start=(ko == 0), stop=(ko == KO - 1),
                        )
                    raw = tpf.tile([P, TCH], bf16, tag="raw")
                    nc.scalar.activation(raw[:], pqk[:], AF.Copy)
                    # half swaps, one per HWDGE queue (bf16: 64KB each)
                    swp = tpw.tile([P, TCH], bf16, tag="swp")
                    nc.sync.dma_start(swp[0:64, :], raw[64:128, :])
                    nc.scalar.dma_start(swp[64:128, :], raw[0:64, :])
                    tmpa = tpw.tile([P, TCH], bf16, tag="tmpa")
                    nc.vector.tensor_mul(tmpa[:], raw[:], cc_sb[:, t0 : t0 + TCH])
                    swps[fb] = swp
                    tmpas[fb] = tmpa

                def emit_fb_b(fb):
                    tmpb = tpf.tile([P, TCH], bf16, tag="tmpb")
                    nc.vector.tensor_mul(tmpb[:], swps[fb][:], ss_sb[:, t0 : t0 + TCH])
                    seg = qk_rt[:, fb, t0 : t0 + TCH]
                    nc.vector.tensor_add(seg, tmpas[fb][:], tmpb[:])
                    sq = sqp.tile([P, TCH], bf16, tag="sq")
                    nc.vector.tensor_mul(sq[:], seg, seg)
                    segs[fb] = (seg, sq)

                def emit_stat(fb):
                    pms = ps_sum.tile([1, TCH], f32, tag="ps_sum")
                    nc.tensor.matmul(pms[:], ones_col[:], segs[fb][1][:], start=True, stop=True)
                    # rstd = 1/sqrt(ms) = sqrt(D / pms); eps negligible vs ms
                    inv = tps.tile([1, TCH], f32, tag="inv")
                    nc.vector.reciprocal_approx_fast(inv[:], pms[:])
                    rstd = rstdp.tile([1, TCH], bf16, tag="rstd")
                    nc.scalar.activation(rstd[:], inv[:], AF.Sqrt, scale=float(D))
                    rstds[fb] = rstd

                def emit_apply(fb):
                    pb = ps_mm.tile([P, TCH], f32, tag="ps_mm")
                    nc.tensor.matmul(pb[:], ones_row[:], rstds[fb][:], start=True, stop=True)
                    seg = segs[fb][0]
                    nc.vector.tensor_mul(seg, seg, pb[:])

                def emit_v(tb):
                    pv = ps_mm.tile([P, TCH], f32, tag="ps_mm")
                    for ko in range(KO):
                        nc.tensor.matmul(
                            pv[:, :FK],
                            xt[:, ko, tb * P : (tb + 1) * P],
                            wv_sb[:, ko, :],
                            start=(ko == 0), stop=(ko == KO - 1),
                        )
                    nc.vector.tensor_copy(
                        v_sb[:, c * (TCH // P) + tb, :], pv[:, :FK]
                    )

                # dense fb block first (max slack for the swap-DMA chains),
                # then V blocks with the stat/apply chains interleaved
                for fb in (4, 5, 0, 1, 2, 3):
                    emit_fb_a(fb)
                pop_deferred()                      # prev apply2
                pop_deferred()                      # prev apply3
                if c + 1 < NCHUNK:
                    issue_x(c + 1, (nc.sync,))
                if c == 1:
                    for h in range(NQ):
                        eng = nc.sync if h % 2 == 0 else nc.scalar
                        eng.dma_start(wo_sb[:, h], wot[h])
                for fb in (4, 5, 0, 1, 2, 3):
                    emit_fb_b(fb)
                emit_v(0)
                emit_stat(4)
                emit_v(1)
                emit_stat(5)
                emit_stat(0)
                emit_v(2)
                emit_stat(1)
                emit_apply(4)
                emit_apply(5)
                emit_v(3)
                emit_stat(2)
                emit_apply(0)
                emit_stat(3)
                emit_apply(1)
                # only the (table-free) applies defer into the next phase, so
                # no Sqrt table load ever lands inside the span exp stream
                deferred = [
                    lambda: emit_apply(2),
                    lambda: emit_apply(3),
                ]
                return deferred

            # ---------------- attention span ----------------
            def emit_norm(ot_t, h, ot_ps, rec_r):
                bc = ps_mm.tile([P, SPAN], f32, tag="ps_mm")
                nc.tensor.matmul(bc[:], ones_row[:], rec_r[:], start=True, stop=True)
                bc_sb = tps.tile([P, SPAN], f32, tag="bc_sb")
                nc.vector.tensor_copy(bc_sb[:], bc[:])
                nc.vector.tensor_mul(ot_t[:, h, :], ot_ps[:], bc_sb[:])

            def emit_span(s, fillers):
                """Attention for q-span s. The two q-heads sharing a kv-head
                advance in interleaved waves; one 2-bank PSUM score tile per
                wave feeds a single batched exp. `fillers`: independent PE
                thunks popped one per wave. Returns (ot_t, deferred norms)."""
                q0 = s * SPAN
                nkb = 4 * s + 4
                ot_t = otp.tile([P, NQ, SPAN], bf16, tag="ot_t")
                pending = []

                for j in range(NK):  # kv head = head pair
                    # free previous pair's ot banks before this pair's AVs;
                    # a filler ahead of each norm covers the DVE rec latency
                    while pending:
                        if fillers:
                            fillers.pop(0)()
                        emit_norm(ot_t, *pending.pop(0))
                    hs = (2 * j, 2 * j + 1)
                    ot_ps = {h: ps_ot.tile([P, SPAN], f32, tag="ot_ps", name="ot_ps")
                             for h in hs}
                    sum_ps = {h: ps_sum.tile([1, SPAN], f32, tag="ps_sum", name="sum_ps")
                              for h in hs}
                    queue = []

                    def flush_one():
                        kb, off, vq, pt2 = queue.pop(0)
                        for i, h in enumerate(hs):
                            nc.tensor.matmul(
                                ot_ps[h][:, off:],
                                v_sb[:, kb, j * D : (j + 1) * D],
                                pt2[:, i, :vq],
                                start=(kb == 0), stop=(kb == nkb - 1),
                                skip_group_check=True,
                            )
                            nc.tensor.matmul(
                                sum_ps[h][:, off:],
                                ones_col[:],
                                pt2[:, i, :vq],
                                start=(kb == 0), stop=(kb == nkb - 1),
                                skip_group_check=True,
                            )

                    for kb in range(nkb):
                        # flushes + filler first: they give the PE work while
                        # the previous wave's exp drains the single st2 buffer
                        while queue and queue[0][0] <= kb - LAG:
                            flush_one()
                        if fillers:
                            fillers.pop(0)()
                        r = kb - 4 * s           # >=0: diagonal block group
                        off = P * r if r > 0 else 0
                        vq = SPAN - off
                        st2 = ps_st.tile([P, 2, SPAN], f32, tag="st2")
                        for i, h in enumerate(hs):
                            nc.tensor.matmul(
                                st2[:, i, :vq],
                                qk_rt[:, NQ + j, kb * P : (kb + 1) * P],
                                qk_rt[:, h, q0 + off : q0 + SPAN],
                                start=True, stop=True,
                            )
                        pt2 = tpt.tile([P, 2, SPAN], bf16, tag="pt2")
                        nc.scalar.activation(pt2[:, :, :vq], st2[:, :, :vq],
                                             AF.Exp, scale=SCALE)
                        if r >= 0:
                            for i in range(2):
                                nc.gpsimd.tensor_mul(
                                    pt2[:, i, :P], pt2[:, i, :P], mask_sb[:])
                        queue.append((kb, off, vq, pt2))
                    while queue:
                        flush_one()
                    # softmax denominators -> reciprocal on DVE; the PE
                    # broadcast is deferred (next pair / next span's stream)
                    for h in hs:
                        rec = tps.tile([1, SPAN], f32, tag="rec")
                        nc.vector.reciprocal_approx_fast(rec[:], sum_ps[h][:])
                        rec_r = tps.tile([1, SPAN], bf16, tag="rec_r")
                        nc.vector.tensor_copy(rec_r[:], rec[:])
                        pending.append((h, ot_ps[h], rec_r))
                # drain any fillers that didn't fit in the wave slots
                while fillers:
                    fillers.pop(0)()
                return ot_t, pending

            def proj_thunks(c, ot_t, split_dma=False):
                """Output projection for span c as half-size PE thunks: a
                first half accumulates heads 0-1, the second finishes 2-3 and
                ships the tile. Finer granules fill span waves better."""
                ypss = {}

                def half_a(tb, nch):
                    yps = ps_mm.tile([P, 512], f32, tag="ps_mm")
                    for h in (0, 1):
                        nc.tensor.matmul(
                            yps[:],
                            ot_t[:, h, tb * P : (tb + 1) * P],
                            wo_sb[:, h, nch * 512 : (nch + 1) * 512],
                            start=(h == 0), stop=False,
                            skip_group_check=True,
                        )
                    ypss[(tb, nch)] = yps

                def half_b(tb, nch):
                    yps = ypss.pop((tb, nch))
                    for h in (2, 3):
                        nc.tensor.matmul(
                            yps[:],
                            ot_t[:, h, tb * P : (tb + 1) * P],
                            wo_sb[:, h, nch * 512 : (nch + 1) * 512],
                            start=False, stop=(h == 3),
                            skip_group_check=True,
                        )
                    ysb = tpy.tile([P, 512], bf16, tag="ysb")
                    nc.vector.tensor_copy(ysb[:], yps[:])
                    rows = slice((4 * c + tb) * P, (4 * c + tb + 1) * P)
                    if split_dma:
                        # halve the final transfers so the kernel tail isn't
                        # gated by one long DMA
                        nc.sync.dma_start(
                            y[rows, nch * 512 : nch * 512 + 256], ysb[:, :256])
                        nc.scalar.dma_start(
                            y[rows, nch * 512 + 256 : (nch + 1) * 512], ysb[:, 256:])
                    else:
                        nc.sync.dma_start(
                            y[rows, nch * 512 : (nch + 1) * 512], ysb[:])
                out = []
                for tb in range(4):
                    for nch in range(C // 512):
                        out.append(lambda tb=tb, nch=nch: half_a(tb, nch))
                        out.append(lambda tb=tb, nch=nch: half_b(tb, nch))
                return out

            # ---------------- program ----------------
            # chunks first (dense PE, HAM-warm); x prefetch happens inside
            # each chunk, after its swap-DMA issues
            d = emit_chunk(0, [])
            d = emit_chunk(1, d)
            d = emit_chunk(2, d)
            d = emit_chunk(3, d)

            # spans; span s-1's output projection rides in span s's stream.
            # chunk3's deferred applies pad a wave into span 0 so their rstd
            # chains are ready.
            noop = lambda: None
            d = [noop] + d[:1] + [noop] + d[1:]
            ot0, n0 = emit_span(0, d)
            f1 = [lambda n=n: emit_norm(ot0, *n) for n in n0] + proj_thunks(0, ot0)
            ot1, n1 = emit_span(1, f1)
            f2 = [lambda n=n: emit_norm(ot1, *n) for n in n1] + proj_thunks(1, ot1)
            ot2, n2 = emit_span(2, f2)
            f3 = [lambda n=n: emit_norm(ot2, *n) for n in n2] + proj_thunks(2, ot2)
            ot3, n3 = emit_span(3, f3)
            for n in n3:
                emit_norm(ot3, *n)
            for t in proj_thunks(3, ot3, split_dma=True):
                t()
    nc.compile()
    return nc


_NC_CACHE = None


def _get_nc():
    global _NC_CACHE
    if _NC_CACHE is None:
        _NC_CACHE = build()
    return _NC_CACHE


def _host_inputs(x, cos, sin, wq, wk, wv, wo):
    """Build the 8 per-core input maps with DMA-contiguous pre-tiled layouts."""
    bft = ml_dtypes.bfloat16
    cosT = np.ascontiguousarray(cos[0, :, 0, :].T).astype(np.float32)  # (64, T)
    sinT = np.ascontiguousarray(sin[0, :, 0, :].T).astype(np.float32)
    cc = np.ascontiguousarray(np.concatenate([cosT, cosT], axis=0)).astype(bft)  # (128, T)
    ss = np.ascontiguousarray(np.concatenate([sinT, -sinT], axis=0)).astype(bft)
    # mask[k, qq] = 1 if qq >= k (within the 128-wide diagonal sub-block)
    qq = np.arange(P)[None, :]
    kk = np.arange(P)[:, None]
    mask = np.ascontiguousarray((qq >= kk).astype(bft))  # (128, 128)

    # xTt[c, g, p, kk, t] = x[b][c*TCH+t, (4g+kk)*P+p]
    xTts = []
    for b in range(2):
        xb = x[b].astype(bft)                            # (T, C)
        a = xb.reshape(NCHUNK, TCH, 4, 4, P)             # [c, t, g, kk, p]
        xTts.append(np.ascontiguousarray(a.transpose(0, 2, 4, 3, 1)))

    wq16 = wq.astype(bft)
    wk16 = wk.astype(bft)
    wv16 = wv.astype(bft)
    wo16 = wo.astype(bft)
    in_maps = []
    for core in range(8):
        b, tp = divmod(core, 4)
        wq_s = wq16[:, tp * FQ : (tp + 1) * FQ]     # (C, FQ)
        wk_s = wk16[:, tp * FK : (tp + 1) * FK]     # (C, FK)
        wv_s = wv16[:, tp * FK : (tp + 1) * FK]
        wo_s = wo16[tp * FQ : (tp + 1) * FQ, :]     # (FQ, C)
        # wqt[fb, p, ko, d] = wq_s[ko*P+p, fb*D+d]
        a = wq_s.reshape(KO, P, NQ, D)
        wqt = np.ascontiguousarray(a.transpose(2, 1, 0, 3))          # (NQ, P, KO, D)
        # wkt[kh, p, ko, d] = wk_s[ko*P+p, kh*D+d]
        a = wk_s.reshape(KO, P, NK, D)
        wkt = np.ascontiguousarray(a.transpose(2, 1, 0, 3))          # (NK, P, KO, D)
        # wvt[p, ko, f] = wv_s[ko*P+p, f]
        a = wv_s.reshape(KO, P, FK)
        wvt = np.ascontiguousarray(a.transpose(1, 0, 2))             # (P, KO, FK)
        # wot[h, p, n] = wo_s[h*D+p, n]
        wot = np.ascontiguousarray(wo_s.reshape(NQ, P, C))
        in_maps.append(
            {
                "xTt": xTts[b],
                "wqt": wqt,
                "wkt": wkt,
                "wvt": wvt,
                "wot": wot,
                "cc": cc,
                "ss": ss,
                "mask": mask,
            }
        )
    return in_maps


def kernel(x, cos, sin, wq, wk, wv, wo, trace=False):
    x = np.asarray(x, dtype=np.float32)
    cos = np.asarray(cos, dtype=np.float32)
    sin = np.asarray(sin, dtype=np.float32)
    wq = np.asarray(wq, dtype=np.float32)
    wk = np.asarray(wk, dtype=np.float32)
    wv = np.asarray(wv, dtype=np.float32)
    wo = np.asarray(wo, dtype=np.float32)

    nc = _get_nc()
    in_maps = _host_inputs(x, cos, sin, wq, wk, wv, wo)
    res = run_bass_kernel_spmd(nc, in_maps, core_ids=list(range(8)), trace=trace)
    out = np.zeros((2, T, C), dtype=np.float32)
    for c in range(8):
        b = c // 4
        out[b] += res.results[c]["y"].astype(np.float32)
    if trace:
        return out, res
    return out


# revision 24
# speedup vs baseline: 1.0185x; 1.0001x over previous
"""Causal self-attention (RoPE + QK-RMSNorm, GQA 16q/8kv) Trainium2 Bass kernel.

Sharding: 8 cores = 2 batch x 4 tensor-parallel. Core c handles batch b=c//4 and
q-heads [4*tp, 4*tp+4), kv-heads [2*tp, 2*tp+2) where tp=c%4. Each core returns a
partial (T, C) output = O_heads @ wo[rows of its heads]; host sums the 4 partials
per batch (the "all-reduce after c_proj").

v7 schedule: all 4 projection chunks run first (dense back-to-back matmuls keep
the PE HAM-warm), then the 4 attention spans. Output projections of span s ride
as PE fillers inside span s+1's score stream. Within a span, the two q-heads
sharing a kv-head advance in interleaved waves; their scores land in one
2-bank PSUM tile so a single exp call covers both heads (the scalar engine is
the span-phase pacer). Rope temporaries are bf16 (2x DVE rate); V copies and
diagonal masks run on the otherwise-idle GPSIMD. DMA transfers are issued
few-and-large on both HWDGE queues (issue rate, not bandwidth, binds startup).
"""
import sys
import math

sys.path.insert(0, "/opt/trn_rl_repo")

import numpy as np
import ml_dtypes
import concourse.bacc as bacc
import concourse.mybir as mybir
import concourse.tile as tile
from concourse.bass_utils import run_bass_kernel_spmd

P = 128
T = 2048
C = 2048
KO = C // P          # 16 contraction tiles
D = 128              # head dim
NQ = 4               # q heads per core
NK = 2               # kv heads per core
NF = NQ + NK         # 6 rope/rms feature blocks (4 q + 2 k)
FQ = NQ * D          # 512
FK = NK * D          # 256
TCH = 512            # chunk / span size
NCHUNK = T // TCH    # 4
SPAN = 512
KB = T // P          # 16 key blocks
SCALE = 1.0 / math.sqrt(D)
LAG = 3              # AV/sum lag (in waves) behind the score stream

f32 = mybir.dt.float32
bf16 = mybir.dt.bfloat16

AF = mybir.ActivationFunctionType


def build():
    nc = bacc.Bacc("TRN2", target_bir_lowering=False)
    # pre-tiled DMA-contiguous input layouts (see _host_inputs)
    xTt = nc.dram_tensor("xTt", (NCHUNK, 4, P, 4, TCH), bf16, kind="ExternalInput")
    wqt = nc.dram_tensor("wqt", (NQ, P, KO, D), bf16, kind="ExternalInput")
    wkt = nc.dram_tensor("wkt", (NK, P, KO, D), bf16, kind="ExternalInput")
    wvt = nc.dram_tensor("wvt", (P, KO, FK), bf16, kind="ExternalInput")
    wot = nc.dram_tensor("wot", (NQ, P, C), bf16, kind="ExternalInput")
    cc = nc.dram_tensor("cc", (P, T), bf16, kind="ExternalInput")    # [cos; cos]
    ss = nc.dram_tensor("ss", (P, T), bf16, kind="ExternalInput")    # [sin; -sin]
    mask = nc.dram_tensor("mask", (P, P), bf16, kind="ExternalInput")  # [k, qq] = qq>=k
    y = nc.dram_tensor("y", (T, C), bf16, kind="ExternalOutput")

    with tile.TileContext(nc) as tc:
        with (
            tc.tile_pool(name="persist", bufs=1) as persist,
            tc.tile_pool(name="otp", bufs=2) as otp,
            tc.tile_pool(name="xp", bufs=2) as xp,
            tc.tile_pool(name="tpf", bufs=3) as tpf,
            tc.tile_pool(name="tpw", bufs=6) as tpw,
            tc.tile_pool(name="tps", bufs=3) as tps,
            tc.tile_pool(name="sqp", bufs=7) as sqp,
            tc.tile_pool(name="rstdp", bufs=8) as rstdp,
            tc.tile_pool(name="tpt", bufs=6) as tpt,
            tc.tile_pool(name="tpy", bufs=6) as tpy,
            tc.tile_pool(name="ps_mm", bufs=2, space="PSUM") as ps_mm,
            tc.tile_pool(name="ps_st", bufs=1, space="PSUM") as ps_st,
            tc.tile_pool(name="ps_ot", bufs=2, space="PSUM") as ps_ot,
            tc.tile_pool(name="ps_sum", bufs=2, space="PSUM") as ps_sum,
        ):
            qk_rt = persist.tile([P, NF, T], bf16, tag="qk_rt")   # roped+normed qT/kT
            v_sb = persist.tile([P, KB, FK], bf16, tag="v_sb")    # V natural [t-part, kb, feat]
            cc_sb = persist.tile([P, T], bf16, tag="cc_sb")
            ss_sb = persist.tile([P, T], bf16, tag="ss_sb")
            mask_sb = persist.tile([P, P], bf16, tag="mask_sb")
            ones_col = persist.tile([P, 1], bf16, tag="ones_col")    # sums lhsT
            ones_row = persist.tile([1, P], bf16, tag="ones_row")    # bcast lhsT
            ones_f32 = persist.tile([P, 1], f32, tag="ones_f32")
            ones_row_f32 = persist.tile([1, P], f32, tag="ones_row_f32")
            wq_sb = persist.tile([P, NQ, KO, D], bf16, tag="wq_sb")
            wk_sb = persist.tile([P, NK, KO, D], bf16, tag="wk_sb")
            wv_sb = persist.tile([P, KO, FK], bf16, tag="wv_sb")
            wo_sb = persist.tile([P, NQ, C], bf16, tag="wo_sb")

            xts = [None] * NCHUNK

            def issue_x(c, engs):
                """Four 512KB group-transfers; HWDGE issue rate binds, so keep
                the count low."""
                xt = xp.tile([P, KO, TCH], bf16, tag="xt")
                for g in range(4):
                    eng = engs[g % len(engs)]
                    eng.dma_start(xt[:, 4 * g : 4 * (g + 1), :], xTt[c, g])
                xts[c] = xt

            # -- startup DMA schedule: first-needed first, split across both
            #    HWDGE issue queues.
            nc.sync.dma_start(wk_sb[:, 0], wkt[0])
            nc.scalar.dma_start(wk_sb[:, 1], wkt[1])
            nc.sync.dma_start(wq_sb[:, 0], wqt[0])
            nc.scalar.dma_start(wq_sb[:, 1], wqt[1])
            issue_x(0, (nc.sync, nc.scalar))
            nc.sync.dma_start(cc_sb[:], cc[:, :])
            nc.scalar.dma_start(ss_sb[:], ss[:, :])
            nc.sync.dma_start(wq_sb[:, 2], wqt[2])
            nc.scalar.dma_start(wq_sb[:, 3], wqt[3])
            nc.sync.dma_start(wv_sb[:], wvt[:, :, :])
            nc.scalar.dma_start(mask_sb[:], mask[:, :])
            nc.vector.memset(ones_f32[:], 1.0)
            nc.vector.memset(ones_row_f32[:], 1.0)
            nc.vector.tensor_copy(ones_col[:], ones_f32[:])
            nc.vector.tensor_copy(ones_row[:], ones_row_f32[:])

            # ---------------- chunk projection ----------------
            def emit_chunk(c, deferred_in):
                """Project chunk c. `deferred_in`: apply thunks from the
                previous chunk. Returns this chunk's deferred thunks."""
                t0 = c * TCH
                xt = xts[c]
                segs = [None] * NF
                rstds = {}
                dq = list(deferred_in)

                def pop_deferred():
                    if dq:
                        dq.pop(0)()

                swps = {}
                tmpas = {}

                def emit_fb_a(fb):
                    """Projection matmuls + rope prologue: raw copy, swap-DMA
                    issue, cos product. Sin-side products batch in emit_fb_b
                    once all swaps are in flight, so neither the DVE nor the
                    scalar queue head-blocks on DMA latency."""
                    if fb < NQ:
                        w_ap = wq_sb[:, fb]
                    else:
                        w_ap = wk_sb[:, fb - NQ]
                    pqk = ps_mm.tile([P, TCH], f32, tag="ps_mm")
                    for ko in range(KO):
                        nc.tensor.matmul(
                            pqk[:], w_ap[:, ko], xt[:, ko, :],
                            start=(ko == 0), stop=(ko == KO - 1),
                        )
                    raw = tpf.tile([P, TCH], bf16, tag="raw")
                    nc.scalar.activation(raw[:], pqk[:], AF.Copy)
                    # half swaps, one per HWDGE queue (bf16: 64KB each)
                    swp = tpw.tile([P, TCH], bf16, tag="swp")
                    nc.sync.dma_start(swp[0:64, :], raw[64:128, :])
                    nc.scalar.dma_start(swp[64:128, :], raw[0:64, :])
                    tmpa = tpw.tile([P, TCH], bf16, tag="tmpa")
                    nc.vector.tensor_mul(tmpa[:], raw[:], cc_sb[:, t0 : t0 + TCH])
                    swps[fb] = swp
                    tmpas[fb] = tmpa

                def emit_fb_b(fb):
                    tmpb = tpf.tile([P, TCH], bf16, tag="tmpb")
                    nc.vector.tensor_mul(tmpb[:], swps[fb][:], ss_sb[:, t0 : t0 + TCH])
                    seg = qk_rt[:, fb, t0 : t0 + TCH]
                    nc.vector.tensor_add(seg, tmpas[fb][:], tmpb[:])
                    sq = sqp.tile([P, TCH], bf16, tag="sq")
                    nc.vector.tensor_mul(sq[:], seg, seg)
                    segs[fb] = (seg, sq)

                def emit_stat(fb):
                    pms = ps_sum.tile([1, TCH], f32, tag="ps_sum")
                    nc.tensor.matmul(pms[:], ones_col[:], segs[fb][1][:], start=True, stop=True)
                    # rstd = 1/sqrt(ms) = sqrt(D / pms); eps negligible vs ms
                    inv = tps.tile([1, TCH], f32, tag="inv")
                    nc.vector.reciprocal_approx_fast(inv[:], pms[:])
                    rstd = rstdp.tile([1, TCH], bf16, tag="rstd")
                    nc.scalar.activation(rstd[:], inv[:], AF.Sqrt, scale=float(D))
                    rstds[fb] = rstd

                def emit_apply(fb):
                    pb = ps_mm.tile([P, TCH], f32, tag="ps_mm")
                    nc.tensor.matmul(pb[:], ones_row[:], rstds[fb][:], start=True, stop=True)
                    seg = segs[fb][0]
                    nc.vector.tensor_mul(seg, seg, pb[:])

                def emit_v(tb):
                    pv = ps_mm.tile([P, TCH], f32, tag="ps_mm")
                    for ko in range(KO):
                        nc.tensor.matmul(
                            pv[:, :FK],
                            xt[:, ko, tb * P : (tb + 1) * P],
                            wv_sb[:, ko, :],
                            start=(ko == 0), stop=(ko == KO - 1),
                        )
                    nc.vector.tensor_copy(
                        v_sb[:, c * (TCH // P) + tb, :], pv[:, :FK]
                    )

                # dense fb block first (max slack for the swap-DMA chains),
                # then V blocks with the stat/apply chains interleaved;
                # deferred thunks from the previous chunk spread through the
                # fb block (their inputs are long since ready)
                for fb in (4, 5, 0, 1, 2, 3):
                    emit_fb_a(fb)
                    pop_deferred()
                pop_deferred()
                pop_deferred()
                if c + 1 < NCHUNK:
                    issue_x(c + 1, (nc.sync,))
                if c == 1:
                    for h in range(NQ):
                        eng = nc.sync if h % 2 == 0 else nc.scalar
                        eng.dma_start(wo_sb[:, h], wot[h])
                for fb in (4, 5, 0, 1, 2, 3):
                    emit_fb_b(fb)
                if c == 0:
                    # chunk 0's input DMA congestion delays its rope chains;
                    # keep only the earliest-ready stats in-chunk
                    emit_v(0)
                    emit_stat(4)
                    emit_v(1)
                    emit_stat(5)
                    emit_stat(0)
                    emit_v(2)
                    emit_apply(4)
                    emit_apply(5)
                    emit_v(3)
                    emit_apply(0)
                    return [lambda fb=fb: emit_stat(fb) for fb in ()] + [
                        t for fb in (1, 2, 3)
                        for t in (lambda fb=fb: emit_stat(fb),
                                  lambda fb=fb: emit_apply(fb))
                    ]
                emit_v(0)
                emit_stat(4)
                emit_v(1)
                emit_stat(5)
                emit_stat(0)
                emit_v(2)
                emit_stat(1)
                emit_apply(4)
                emit_apply(5)
                emit_v(3)
                emit_stat(2)
                emit_apply(0)
                emit_stat(3)
                emit_apply(1)
                # only the (table-free) applies defer into the next phase, so
                # no Sqrt table load ever lands inside the span exp stream
                deferred = [
                    lambda: emit_apply(2),
                    lambda: emit_apply(3),
                ]
                return deferred

            # ---------------- attention span ----------------
            def emit_norm(ot_t, h, ot_ps, rec_r):
                bc = ps_mm.tile([P, SPAN], f32, tag="ps_mm")
                nc.tensor.matmul(bc[:], ones_row[:], rec_r[:], start=True, stop=True)
                bc_sb = tps.tile([P, SPAN], f32, tag="bc_sb")
                nc.vector.tensor_copy(bc_sb[:], bc[:])
                nc.vector.tensor_mul(ot_t[:, h, :], ot_ps[:], bc_sb[:])

            def emit_span(s, fillers):
                """Attention for q-span s. The two q-heads sharing a kv-head
                advance in interleaved waves; one 2-bank PSUM score tile per
                wave feeds a single batched exp. `fillers`: independent PE
                thunks popped one per wave. Returns (ot_t, deferred norms)."""
                q0 = s * SPAN
                nkb = 4 * s + 4
                ot_t = otp.tile([P, NQ, SPAN], bf16, tag="ot_t")
                pending = []

                for j in range(NK):  # kv head = head pair
                    # free previous pair's ot banks before this pair's AVs;
                    # a filler ahead of each norm covers the DVE rec latency
                    while pending:
                        if fillers:
                            fillers.pop(0)()
                        emit_norm(ot_t, *pending.pop(0))
                    hs = (2 * j, 2 * j + 1)
                    ot_ps = {h: ps_ot.tile([P, SPAN], f32, tag="ot_ps", name="ot_ps")
                             for h in hs}
                    sum_ps = {h: ps_sum.tile([1, SPAN], f32, tag="ps_sum", name="sum_ps")
                              for h in hs}
                    queue = []

                    def flush_one():
                        kb, off, vq, pt2 = queue.pop(0)
                        for i, h in enumerate(hs):
                            nc.tensor.matmul(
                                ot_ps[h][:, off:],
                                v_sb[:, kb, j * D : (j + 1) * D],
                                pt2[:, i, :vq],
                                start=(kb == 0), stop=(kb == nkb - 1),
                                skip_group_check=True,
                            )
                            nc.tensor.matmul(
                                sum_ps[h][:, off:],
                                ones_col[:],
                                pt2[:, i, :vq],
                                start=(kb == 0), stop=(kb == nkb - 1),
                                skip_group_check=True,
                            )

                    for kb in range(nkb):
                        # flushes + filler first: they give the PE work while
                        # the previous wave's exp drains the single st2 buffer
                        while queue and queue[0][0] <= kb - LAG:
                            flush_one()
                        if fillers:
                            fillers.pop(0)()
                        r = kb - 4 * s           # >=0: diagonal block group
                        off = P * r if r > 0 else 0
                        vq = SPAN - off
                        st2 = ps_st.tile([P, 2, SPAN], f32, tag="st2")
                        for i, h in enumerate(hs):
                            nc.tensor.matmul(
                                st2[:, i, :vq],
                                qk_rt[:, NQ + j, kb * P : (kb + 1) * P],
                                qk_rt[:, h, q0 + off : q0 + SPAN],
                                start=True, stop=True,
                            )
                        pt2 = tpt.tile([P, 2, SPAN], bf16, tag="pt2")
                        nc.scalar.activation(pt2[:, :, :vq], st2[:, :, :vq],
                                             AF.Exp, scale=SCALE)
                        if r >= 0:
                            for i in range(2):
                                nc.gpsimd.tensor_mul(
                                    pt2[:, i, :P], pt2[:, i, :P], mask_sb[:])
                        queue.append((kb, off, vq, pt2))
                    while queue:
                        flush_one()
                        if fillers:
                            fillers.pop(0)()
                    # softmax denominators -> reciprocal on DVE; the PE
                    # broadcast is deferred (next pair / next span's stream)
                    for h in hs:
                        rec = tps.tile([1, SPAN], f32, tag="rec")
                        nc.vector.reciprocal_approx_fast(rec[:], sum_ps[h][:])
                        rec_r = tps.tile([1, SPAN], bf16, tag="rec_r")
                        nc.vector.tensor_copy(rec_r[:], rec[:])
                        pending.append((h, ot_ps[h], rec_r))
                # drain any fillers that didn't fit in the wave slots
                while fillers:
                    fillers.pop(0)()
                return ot_t, pending

            def proj_thunks(c, ot_t, split_dma=False):
                """Output projection for span c as half-size PE thunks: a
                first half accumulates heads 0-1, the second finishes 2-3 and
                ships the tile. Finer granules fill span waves better."""
                ypss = {}

                def half_a(tb, nch):
                    yps = ps_mm.tile([P, 512], f32, tag="ps_mm")
                    for h in (0, 1):
                        nc.tensor.matmul(
                            yps[:],
                            ot_t[:, h, tb * P : (tb + 1) * P],
                            wo_sb[:, h, nch * 512 : (nch + 1) * 512],
                            start=(h == 0), stop=False,
                            skip_group_check=True,
                        )
                    ypss[(tb, nch)] = yps

                def half_b(tb, nch):
                    yps = ypss.pop((tb, nch))
                    for h in (2, 3):
                        nc.tensor.matmul(
                            yps[:],
                            ot_t[:, h, tb * P : (tb + 1) * P],
                            wo_sb[:, h, nch * 512 : (nch + 1) * 512],
                            start=False, stop=(h == 3),
                            skip_group_check=True,
                        )
                    ysb = tpy.tile([P, 512], bf16, tag="ysb")
                    nc.vector.tensor_copy(ysb[:], yps[:])
                    rows = slice((4 * c + tb) * P, (4 * c + tb + 1) * P)
                    if split_dma:
                        # halve the final transfers so the kernel tail isn't
                        # gated by one long DMA
                        nc.sync.dma_start(
                            y[rows, nch * 512 : nch * 512 + 256], ysb[:, :256])
                        nc.scalar.dma_start(
                            y[rows, nch * 512 + 256 : (nch + 1) * 512], ysb[:, 256:])
                    else:
                        nc.sync.dma_start(
                            y[rows, nch * 512 : (nch + 1) * 512], ysb[:])
                out = []
                for tb in range(4):
                    for nch in range(C // 512):
                        out.append(lambda tb=tb, nch=nch: half_a(tb, nch))
                        out.append(lambda tb=tb, nch=nch: half_b(tb, nch))
                return out

            # ---------------- program ----------------
            # chunks first (dense PE, HAM-warm); x prefetch happens inside
            # each chunk, after its swap-DMA issues
            d = emit_chunk(0, [])
            d = emit_chunk(1, d)
            d = emit_chunk(2, d)
            d = emit_chunk(3, d)

            # spans; span s-1's output projection rides in span s's stream.
            # chunk3's deferred applies pad a wave into span 0 so their rstd
            # chains are ready.
            noop = lambda: None
            d = [noop] + d[:1] + [noop] + d[1:]
            ot0, n0 = emit_span(0, d)
            f1 = [lambda n=n: emit_norm(ot0, *n) for n in n0] + proj_thunks(0, ot0)
            ot1, n1 = emit_span(1, f1)
            f2 = [lambda n=n: emit_norm(ot1, *n) for n in n1] + proj_thunks(1, ot1)
            ot2, n2 = emit_span(2, f2)
            f3 = [lambda n=n: emit_norm(ot2, *n) for n in n2] + proj_thunks(2, ot2)
            ot3, n3 = emit_span(3, f3)
            for n in n3:
                emit_norm(ot3, *n)
            for t in proj_thunks(3, ot3, split_dma=True):
                t()
    nc.compile()
    return nc


_NC_CACHE = None


def _get_nc():
    global _NC_CACHE
    if _NC_CACHE is None:
        _NC_CACHE = build()
    return _NC_CACHE


def _host_inputs(x, cos, sin, wq, wk, wv, wo):
    """Build the 8 per-core input maps with DMA-contiguous pre-tiled layouts."""
    bft = ml_dtypes.bfloat16
    cosT = np.ascontiguousarray(cos[0, :, 0, :].T).astype(np.float32)  # (64, T)
    sinT = np.ascontiguousarray(sin[0, :, 0, :].T).astype(np.float32)
    cc = np.ascontiguousarray(np.concatenate([cosT, cosT], axis=0)).astype(bft)  # (128, T)
    ss = np.ascontiguousarray(np.concatenate([sinT, -sinT], axis=0)).astype(bft)
    # mask[k, qq] = 1 if qq >= k (within the 128-wide diagonal sub-block)
    qq = np.arange(P)[None, :]
    kk = np.arange(P)[:, None]
    mask = np.ascontiguousarray((qq >= kk).astype(bft))  # (128, 128)

    # xTt[c, g, p, kk, t] = x[b][c*TCH+t, (4g+kk)*P+p]
    xTts = []
    for b in range(2):
        xb = x[b].astype(bft)                            # (T, C)
        a = xb.reshape(NCHUNK, TCH, 4, 4, P)             # [c, t, g, kk, p]
        xTts.append(np.ascontiguousarray(a.transpose(0, 2, 4, 3, 1)))

    wq16 = wq.astype(bft)
    wk16 = wk.astype(bft)
    wv16 = wv.astype(bft)
    wo16 = wo.astype(bft)
    in_maps = []
    for core in range(8):
        b, tp = divmod(core, 4)
        wq_s = wq16[:, tp * FQ : (tp + 1) * FQ]     # (C, FQ)
        wk_s = wk16[:, tp * FK : (tp + 1) * FK]     # (C, FK)
        wv_s = wv16[:, tp * FK : (tp + 1) * FK]
        wo_s = wo16[tp * FQ : (tp + 1) * FQ, :]     # (FQ, C)
        # wqt[fb, p, ko, d] = wq_s[ko*P+p, fb*D+d]
        a = wq_s.reshape(KO, P, NQ, D)
        wqt = np.ascontiguousarray(a.transpose(2, 1, 0, 3))          # (NQ, P, KO, D)
        # wkt[kh, p, ko, d] = wk_s[ko*P+p, kh*D+d]
        a = wk_s.reshape(KO, P, NK, D)
        wkt = np.ascontiguousarray(a.transpose(2, 1, 0, 3))          # (NK, P, KO, D)
        # wvt[p, ko, f] = wv_s[ko*P+p, f]
        a = wv_s.reshape(KO, P, FK)
        wvt = np.ascontiguousarray(a.transpose(1, 0, 2))             # (P, KO, FK)
        # wot[h, p, n] = wo_s[h*D+p, n]
        wot = np.ascontiguousarray(wo_s.reshape(NQ, P, C))
        in_maps.append(
            {
                "xTt": xTts[b],
                "wqt": wqt,
                "wkt": wkt,
                "wvt": wvt,
                "wot": wot,
                "cc": cc,
                "ss": ss,
                "mask": mask,
            }
        )
    return in_maps


def kernel(x, cos, sin, wq, wk, wv, wo, trace=False):
    x = np.asarray(x, dtype=np.float32)
    cos = np.asarray(cos, dtype=np.float32)
    sin = np.asarray(sin, dtype=np.float32)
    wq = np.asarray(wq, dtype=np.float32)
    wk = np.asarray(wk, dtype=np.float32)
    wv = np.asarray(wv, dtype=np.float32)
    wo = np.asarray(wo, dtype=np.float32)

    nc = _get_nc()
    in_maps = _host_inputs(x, cos, sin, wq, wk, wv, wo)
    res = run_bass_kernel_spmd(nc, in_maps, core_ids=list(range(8)), trace=trace)
    out = np.zeros((2, T, C), dtype=np.float32)
    for c in range(8):
        b = c // 4
        out[b] += res.results[c]["y"].astype(np.float32)
    if trace:
        return out, res
    return out
